# revision 10
# baseline (speedup 1.0000x reference)
"""Self-contained Trainium2 Bass kernel for nn_AttnBlock (VAE-style attention).

Reference computation (per batch b):
  hn = GroupNorm32(x)                      # [C, N], stats per group of 16 chans
  q/k/v = W @ hn + b                       # 1x1 convs, C=512
  attn = softmax(q^T k / sqrt(C), axis=j)  # N=4096 spatial positions
  out  = x + Wp @ (v @ attn^T) + bp

Sharding: 8 cores = 2 batches x 4 query chunks of 1024. Each core receives
its batch's full image ROLLED so its local 1024 query columns come first,
making the SPMD program identical on every core (key order under softmax is
permutation invariant). GroupNorm + keys/values cover the full image on each
core; queries/residual only the local chunk.

Weight fusions (host-precomputed, all exact algebra):
  W2 = k_w^T (s q_w), b2 = k_w^T (s q_b):  scores^T = hn^T (W2 hn + b2)
       (the per-query offset bk.q is softmax-invariant and dropped) — K is
       never materialized.
  W3 = proj_w v_w:  since column scaling by 1/den commutes through left
       matmul, out = (W3 hn E) * recip_den — V and proj collapse into one.
  bp_eff = proj_w v_b + proj_b.

fp8 DoubleRow acceleration (tolerance is 2e-2; this lands ~4e-3):
  All four big matmul groups (q2, vt3, scores, AV) run as float8e4
  DoubleRow matmuls: contraction 256 deep per instruction at the same
  per-instruction cost as a 128-deep fp32r matmul (measured ~280 ns per
  [256]x[128x512] MM) — 2x the MAC rate, halving the MM count vs fp32r.
  Scale folding keeps everything in fp8's sweet spot and is exact:
    w28 = fp8(AW2*W2) so scores_psum = AW2*scores; exp applies scale 1/AW2.
    Per-key exp shift of EXP_SHIFT keeps E=exp(s+shift) <= ~40 < 240 (fp8e4
    max); the shift cancels between numerator and denominator.
    w38 = fp8(AV3*W3) so vt3/AV carry AV3; the ones-vector of the
    denominator cross-partition sum matmul is AV3 instead of 1, so the
    reciprocal cancels AV3 exactly.
  GroupNorm stats run on f32 x (resident, also reused for the residual);
  the normalize step writes hn directly as fp8 (never materialized f32).

Per-core dataflow (c = channel, j = key pos, i = query pos):
  hn8a/hn8b [c-part, 2-slab, n] fp8 — slab pairs (0,1) and (2,3) giving the
  256-deep DoubleRow contraction layout.
  q28 = fp8(W28 hn + AW2 b2)  [c-part, pair, slab, i-local]
  vt38 = fp8(hn^T W38^T)      [j-part, 16 chunk-pair, 2, c]
  per query tile of 512 (outU resident in 4 PSUM banks across the key loop):
    per key chunk PAIR of 256 (2 x 128):
      scoresT pair = hn8^T q28   [j-part, 2, i] one 2-bank PSUM tile
      E8 = exp(scale*psum+shift) ONE ScalarE activation over 1024 cols, fp8
      den partials                (DVE even / Pool odd pairs)
      outU += vt38^T E8          4 DoubleRow matmuls accumulating in PSUM
    y = outU * recip(AV3*den) + bp_eff + x_local   (x still resident)
~342 matmuls total vs 678 fp32r in the prior version.
"""

import numpy as np
import ml_dtypes

import concourse.bass as bass
import concourse.mybir as mybir
from concourse import bacc
import concourse.tile as tile
from concourse import bass_utils

P = 128          # partitions
C = 512          # channels
CS = C // P      # channel slabs (4)
G = 32           # groups
GS = C // G      # channels per group (16)
EPS = 1e-6
F32 = mybir.dt.float32
F32R = mybir.dt.float32r
FP8 = mybir.dt.float8e4
DR = mybir.MatmulPerfMode.DoubleRow
AL = mybir.AluOpType
AF = mybir.ActivationFunctionType

N_FULL = 4096    # spatial positions (64*64)
NQ = 1024        # local query chunk per core
IT = 512         # i-tile (queries per scores matmul free dim)

AW2 = 64.0       # fp8 range scale folded into W2 (and b2); exp scale 1/AW2
AV3 = 8.0        # fp8 range scale folded into W3; cancelled via ones=AV3
EXP_SHIFT = -2.0  # exp(s + shift): keeps E below fp8e4 max; cancels in ratio


def build_nc(n=N_FULL, nq=NQ, repeat=1, ablate=()):
    """Build the per-core Bass program. All 8 cores run this same program."""
    njc = n // P          # 32 key chunks
    npr = njc // 2        # 16 key chunk pairs
    nit = nq // IT        # 2 query tiles
    ablate = set(ablate)

    nc = bacc.Bacc("TRN2", target_bir_lowering=False, debug=False)

    x_d = nc.dram_tensor("x", [C, n], F32, kind="ExternalInput")
    w28_d = nc.dram_tensor("w28", [P, 2, 2, C], FP8, kind="ExternalInput")
    w38_d = nc.dram_tensor("w38", [P, 2, 2, C], FP8, kind="ExternalInput")
    # per partition p: [bq(CS), bp(CS), gamma(CS), beta(CS), bo_row(P), onv_row(P)]
    cpk_d = nc.dram_tensor("cpk", [P, 4 * CS + 2 * P], F32, kind="ExternalInput")
    y_d = nc.dram_tensor("y", [C, nq], F32, kind="ExternalOutput")

    # [C, n] viewed as [P, CS, n]: channel c = slab*128 + partition
    x_t = x_d.rearrange("(o p) n -> p o n", p=P)
    y_t = y_d.rearrange("(o p) n -> p o n", p=P)

    with tile.TileContext(nc) as tc:
        with (
            tc.tile_pool(name="consts", bufs=1) as consts,
            tc.tile_pool(name="xp", bufs=1) as xp,
            tc.tile_pool(name="hp", bufs=1) as hp,
            tc.tile_pool(name="persist", bufs=1) as persist,
            tc.tile_pool(name="ep", bufs=6) as ep,
            tc.tile_pool(name="dt", bufs=4) as dtp,
            tc.tile_pool(name="xres", bufs=3) as xres,
            tc.tile_pool(name="yp", bufs=2) as yp,
            tc.tile_pool(name="psmm", bufs=4, space="PSUM") as psmm,
            tc.tile_pool(name="psacc", bufs=4, space="PSUM") as psacc,
        ):
            # ---- constants (outside the repeat loop) ----
            w28_sb = consts.tile([P, 2, 2, C], FP8, tag="w28")
            w38_sb = consts.tile([P, 2, 2, C], FP8, tag="w38")
            nc.scalar.dma_start(out=w28_sb, in_=w28_d[:, :, :, :])
            nc.scalar.dma_start(out=w38_sb, in_=w38_d[:, :, :, :])
            cpk_sb = consts.tile([P, 4 * CS + 2 * P], F32, tag="cpk")
            nc.scalar.dma_start(out=cpk_sb, in_=cpk_d[:, :])
            bq_sb = cpk_sb[:, 0 * CS:1 * CS]
            bp_sb = cpk_sb[:, 1 * CS:2 * CS]
            gam_sb = cpk_sb[:, 2 * CS:3 * CS]
            bet_sb = cpk_sb[:, 3 * CS:4 * CS]
            bo_sb = cpk_sb[:, 4 * CS:4 * CS + P]
            onv_sb = cpk_sb[:, 4 * CS + P:4 * CS + 2 * P]
            e8c_sb = consts.tile([P, 2, IT], FP8, tag="e8c")
            nc.vector.memset(e8c_sb, 1.0)
            eps_sb = consts.tile([P, 1], F32, tag="eps")
            nc.vector.memset(eps_sb, EPS)
            sh_sb = consts.tile([P, 1], F32, tag="sh")
            nc.vector.memset(sh_sb, EXP_SHIFT)

            HN = n // 2   # x slabs live as two half tiles for DMA pipelining

            def body():
                # ---- phase 1: load x, groupnorm per slab (groups of 16
                # channels never cross a 128-channel slab). x stays resident
                # f32 (reused for the residual); hn is written as fp8 into
                # the DoubleRow slab-pair layout.
                x_sbs = []
                hn8 = [hp.tile([P, 2, n], FP8, tag=f"hn8{pr}", name=f"hn8{pr}")
                       for pr in range(2)]
                for po in range(CS):
                    parts = []
                    engs = [nc.sync, nc.gpsimd]
                    for hh in range(2):
                        xsh = xp.tile([P, HN], F32, tag=f"x{po}_{hh}",
                                      name=f"x{po}_{hh}")
                        engs[hh].dma_start(
                            out=xsh, in_=x_t[:, po, hh * HN:(hh + 1) * HN])
                        parts.append(xsh)
                    x_sbs.append(parts)
                    if "nogn" in ablate:
                        scl = consts.tile([P, 1], F32, tag=f"scl{po}",
                                          name=f"scl{po}")
                        shf = consts.tile([P, 1], F32, tag=f"shf{po}",
                                          name=f"shf{po}")
                        nc.vector.memset(scl, 1.0)
                        nc.vector.memset(shf, 0.0)
                        if "nonorm" not in ablate:
                            for hh in range(2):
                                nc.vector.tensor_scalar(
                                    out=hn8[po // 2][:, po % 2,
                                                     hh * HN:(hh + 1) * HN],
                                    in0=parts[hh], scalar1=scl, scalar2=shf,
                                    op0=AL.mult, op1=AL.subtract)
                        continue
                    nchunk = n // 512
                    nch = nchunk // 2
                    stats = consts.tile([P, nchunk, 6], F32, tag=f"st{po}",
                                        name=f"st{po}")
                    for hh in range(2):
                        xs3 = parts[hh].rearrange("p (s f) -> p s f", f=512)
                        for s in range(nch):
                            nc.vector.bn_stats(out=stats[:, hh * nch + s, :],
                                               in_=xs3[:, s, :])
                    mv = consts.tile([P, 2], F32, tag=f"mv{po}", name=f"mv{po}")
                    nc.vector.bn_aggr(out=mv, in_=stats)
                    # var -> E[x^2] = mean*mean + var (in place)
                    nc.vector.scalar_tensor_tensor(
                        out=mv[:, 1:2], in0=mv[:, 0:1], scalar=mv[:, 0:1],
                        in1=mv[:, 1:2], op0=AL.mult, op1=AL.add)
                    # group-average within the slab: [P, 2] = BO^T @ mv
                    ps_st = psmm.tile([P, IT], F32, tag="ps_mm",
                                      name="ps_st")
                    nc.tensor.matmul(ps_st[:, 0:2], bo_sb, mv,
                                     start=True, stop=True)
                    mvg = consts.tile([P, 2], F32, tag=f"mvg{po}",
                                      name=f"mvg{po}")
                    nc.vector.tensor_copy(out=mvg, in_=ps_st[:, 0:2])
                    gmean = mvg[:, 0:1]   # group E[x] per channel
                    gex2 = mvg[:, 1:2]    # group E[x^2] per channel
                    scl = consts.tile([P, 1], F32, tag=f"scl{po}",
                                      name=f"scl{po}")
                    shf = consts.tile([P, 1], F32, tag=f"shf{po}",
                                      name=f"shf{po}")
                    # scl <- -var = mean^2 - E[x^2]
                    nc.vector.scalar_tensor_tensor(
                        out=scl, in0=gmean, scalar=gmean, in1=gex2,
                        op0=AL.mult, op1=AL.subtract)
                    # sqrt(var + eps) via activation scale=-1
                    nc.scalar.activation(out=scl, in_=scl, func=AF.Sqrt,
                                         bias=eps_sb, scale=-1.0)
                    nc.vector.reciprocal(out=scl, in_=scl)
                    nc.vector.tensor_mul(out=scl, in0=scl,
                                         in1=gam_sb[:, po:po + 1])
                    # shf <- gmean*scl - beta = -(true shift)
                    nc.vector.scalar_tensor_tensor(
                        out=shf, in0=gmean, scalar=scl,
                        in1=bet_sb[:, po:po + 1], op0=AL.mult, op1=AL.subtract)
                    # hn8 = fp8(x*scl - shf), straight into slab-pair layout
                    for hh in ([] if "nonorm" in ablate else range(2)):
                        nc.vector.tensor_scalar(
                            out=hn8[po // 2][:, po % 2,
                                             hh * HN:(hh + 1) * HN],
                            in0=parts[hh], scalar1=scl, scalar2=shf,
                            op0=AL.mult, op1=AL.subtract)

                # ---- phase 2: q28 = fp8(W28 @ hn + AW2*b2) for local
                # queries. One 2-bank PSUM tile per output chunk cc, then a
                # single wide activation applies bias and quantizes.
                q28 = persist.tile([P, 2, 2, nq], FP8, tag="q28", name="q28")
                for cc in range(CS):
                    for it in range(nit):
                        isl = slice(it * IT, (it + 1) * IT)
                        psq = psmm.tile([P, IT], F32, tag="ps_mm", name="ps_q")
                        for pr in range(2):
                            nc.tensor.matmul(
                                psq,
                                w28_sb[:, pr, :, cc * P:(cc + 1) * P],
                                hn8[pr][:, :, isl],
                                start=(pr == 0), stop=(pr == 1),
                                perf_mode=DR)
                        nc.vector.tensor_scalar_add(
                            out=q28[:, cc // 2, cc % 2, isl],
                            in0=psq, scalar1=bq_sb[:, cc:cc + 1])

                # ---- phase 2b: vt38 = fp8(hn^T W38^T), all key chunks,
                # DoubleRow layout [j-part, chunk-pair, 2, C] ----
                vt38 = persist.tile([P, npr, 2, C], FP8, tag="vt38",
                                    name="vt38")
                for jcg in range(njc):
                    psv = psmm.tile([P, IT], F32, tag="ps_mm", name="ps_v")
                    js = jcg * P
                    for pr in range(2):
                        nc.tensor.matmul(
                            psv,
                            hn8[pr][:, :, js:js + P],
                            w38_sb[:, pr, :, :],
                            start=(pr == 0), stop=(pr == 1),
                            perf_mode=DR)
                    if jcg % 2 == 0:
                        nc.vector.tensor_copy(
                            out=vt38[:, jcg // 2, jcg % 2, :], in_=psv)
                    else:
                        nc.scalar.copy(
                            out=vt38[:, jcg // 2, jcg % 2, :], in_=psv)

                # ---- phase 3: per query tile, one pass over all key chunk
                # pairs with the output accumulating in PSUM the whole way ----
                for it in range(nit):
                    isl = slice(it * IT, (it + 1) * IT)
                    dens = [persist.tile([P, IT], F32, tag=f"den{a}{it}",
                                         name=f"den{a}{it}")
                            for a in range(4)]
                    pos = [psacc.tile([P, IT], F32, tag="po", name=f"po{cc}")
                           for cc in range(CS)]
                    for t in range(npr):
                        e8 = ep.tile([P, 2, IT], FP8, tag="e", name="e")
                        for q in range(2):
                            js = (2 * t + q) * P
                            pss = psmm.tile([P, IT], F32, tag="ps_mm",
                                            name="ps_s")
                            if "noscore" not in ablate:
                                for pr in range(2):
                                    nc.tensor.matmul(
                                        pss,
                                        hn8[pr][:, :, js:js + P],
                                        q28[:, pr, :, isl],
                                        start=(pr == 0), stop=(pr == 1),
                                        perf_mode=DR)
                            else:
                                nc.tensor.matmul(
                                    pss, hn8[0][:, :, 0:P],
                                    q28[:, 0, :, isl], start=True, stop=True,
                                    perf_mode=DR)
                            if "noexp2" not in ablate:
                                nc.scalar.activation(
                                    out=e8[:, q, :], in_=pss,
                                    func=(AF.Copy if "noexp" in ablate
                                          else AF.Exp),
                                    scale=1.0 / AW2, bias=sh_sb)
                        if "noexp2" in ablate:
                            e8 = e8c_sb
                        # denominator partials: 4 accumulators, DVE takes
                        # 1/4 of pairs, Pool 3/4 (two chains each)
                        if "noden" in ablate:
                            if t == 0:
                                for a in range(4):
                                    nc.vector.memset(dens[a], 1.0)
                        else:
                            eng = nc.vector if t % 4 == 0 else nc.gpsimd
                            acc = (0 if t % 4 == 0 else
                                   1 + (t - 1 - t // 4) % 3)
                            deng = dens[acc]
                            if t < 4:
                                eng.tensor_tensor(out=deng, in0=e8[:, 0, :],
                                                  in1=e8[:, 1, :], op=AL.add)
                            else:
                                tmp = dtp.tile([P, IT], F32, tag="dt", name="dt")
                                eng.tensor_tensor(out=tmp, in0=e8[:, 0, :],
                                                  in1=e8[:, 1, :], op=AL.add)
                                eng.tensor_add(out=deng, in0=deng, in1=tmp)
                        # AV accumulate into psum across the whole key loop
                        if "noav" not in ablate or t in (0, npr - 1):
                            for cc in range(CS):
                                nc.tensor.matmul(
                                    pos[cc],
                                    vt38[:, t, :, cc * P:(cc + 1) * P],
                                    e8,
                                    start=(t == 0), stop=(t == npr - 1),
                                    perf_mode=DR)

                    # tail: y = pos*recip + bp_eff + x  (recip via one
                    # AV3-valued ones matmul: cross-partition sum + bcast)
                    nc.vector.tensor_add(out=dens[0], in0=dens[0], in1=dens[1])
                    nc.gpsimd.tensor_add(out=dens[2], in0=dens[2], in1=dens[3])
                    nc.vector.tensor_add(out=dens[0], in0=dens[0], in1=dens[2])
                    ps_d = psmm.tile([P, IT], F32, tag="ps_mm", name="ps_d")
                    nc.tensor.matmul(ps_d, onv_sb, dens[0],
                                     start=True, stop=True)
                    recip = consts.tile([P, IT], F32, tag=f"recip{it}",
                                        name=f"recip{it}")
                    nc.vector.reciprocal(out=recip, in_=ps_d)
                    for cc in range(CS):
                        yt = yp.tile([P, IT], F32, tag="yt", name="yt")
                        xr = xres.tile([P, IT], F32, tag="xr", name="xr")
                        nc.gpsimd.dma_start(out=xr, in_=x_t[:, cc, isl])
                        nc.vector.tensor_tensor(
                            out=yt, in0=pos[cc], in1=recip, op=AL.mult)
                        nc.vector.scalar_tensor_tensor(
                            out=yt, in0=yt, scalar=bp_sb[:, cc:cc + 1],
                            in1=xr, op0=AL.add, op1=AL.add)
                        nc.scalar.dma_start(out=y_t[:, cc, isl], in_=yt)

            if repeat == 1:
                body()
            else:
                with tc.For_i(0, repeat, 1):
                    body()

    nc.compile()
    return nc


_NC_CACHE = {}


def _get_nc(n=N_FULL, nq=NQ, repeat=1, ablate=()):
    key = (n, nq, repeat, tuple(sorted(ablate)))
    if key not in _NC_CACHE:
        _NC_CACHE[key] = build_nc(n, nq, repeat, ablate)
    return _NC_CACHE[key]


def _fp8(a):
    return np.clip(a, -240.0, 240.0).astype(ml_dtypes.float8_e4m3)


def make_in_maps(x, q_w, q_b, k_w, k_b, v_w, v_b, proj_w, proj_b,
                 norm_gamma, norm_beta, n_cores=8):
    """Build per-core input dicts from the full problem inputs."""
    B = x.shape[0]
    n = x.shape[2] * x.shape[3]
    xf = np.ascontiguousarray(x.reshape(B, C, n).astype(np.float32))
    scale = np.float64(C) ** -0.5
    # fused score projection: scores^T = hn^T @ (W2 hn + b2) (+ const per
    # query, dropped — softmax invariant)
    W2 = k_w.astype(np.float64).T @ (q_w.astype(np.float64) * scale)
    b2 = k_w.astype(np.float64).T @ (q_b.astype(np.float64) * scale)
    W3 = proj_w.astype(np.float64) @ v_w.astype(np.float64)
    bp_eff = (proj_w.astype(np.float64) @ v_b.astype(np.float64)
              + proj_b.astype(np.float64)).astype(np.float32)

    def pack_dr(W, s):  # [C_out, C_in] -> fp8 [P, 2, 2, C_out], scaled
        Wt = np.ascontiguousarray((W.T * s).astype(np.float32))  # [C_in, C_out]
        return _fp8(Wt.reshape(2, 2, P, C).transpose(2, 0, 1, 3))

    w28 = np.ascontiguousarray(pack_dr(W2, AW2))
    w38 = np.ascontiguousarray(pack_dr(W3, AV3))
    # block-diagonal group-averaging matrix: 16x16 blocks of 1/16
    bo = np.zeros((P, P), np.float32)
    for g in range(P // GS):
        bo[g * GS:(g + 1) * GS, g * GS:(g + 1) * GS] = 1.0 / GS
    onv = np.full((P, P), AV3, np.float32)

    def r2h(v):  # [C] -> [P, CS] with c = o*P + p
        return np.ascontiguousarray(
            np.asarray(v, np.float64).reshape(CS, P).T.astype(np.float32))
    cpk = np.concatenate(
        [r2h(b2 * AW2), r2h(bp_eff),
         r2h(norm_gamma.astype(np.float32)), r2h(norm_beta.astype(np.float32)),
         bo, onv], axis=1)
    chunks = n_cores // B
    nq = n // chunks
    in_maps = []
    for g in range(n_cores):
        b, qc = divmod(g, chunks)
        xg = np.roll(xf[b], -qc * nq, axis=1)
        in_maps.append(dict(
            x=np.ascontiguousarray(xg), w28=w28, w38=w38, cpk=cpk))
    return in_maps


def kernel(**inputs):
    x = np.asarray(inputs["x"], np.float32)
    B, _, H, W = x.shape
    n = H * W
    chunks = 8 // B
    nq = n // chunks
    in_maps = make_in_maps(
        x, np.asarray(inputs["q_w"]), np.asarray(inputs["q_b"]),
        np.asarray(inputs["k_w"]), np.asarray(inputs["k_b"]),
        np.asarray(inputs["v_w"]), np.asarray(inputs["v_b"]),
        np.asarray(inputs["proj_w"]), np.asarray(inputs["proj_b"]),
        np.asarray(inputs["norm_gamma"]), np.asarray(inputs["norm_beta"]))
    nc = _get_nc(n, nq)
    res = bass_utils.run_bass_kernel_spmd(nc, in_maps, core_ids=list(range(8)))
    y = np.empty((B, C, n), np.float32)
    for g in range(8):
        b, qc = divmod(g, chunks)
        y[b][:, qc * nq:(qc + 1) * nq] = res.results[g]["y"]
    return y.reshape(B, C, H, W)


# revision 11
# speedup vs baseline: 1.1654x; 1.1654x over previous
"""Self-contained Trainium2 Bass kernel for nn_AttnBlock (VAE-style attention).

Reference computation (per batch b):
  hn = GroupNorm32(x)                      # [C, N], stats per group of 16 chans
  q/k/v = W @ hn + b                       # 1x1 convs, C=512
  attn = softmax(q^T k / sqrt(C), axis=j)  # N=4096 spatial positions
  out  = x + Wp @ (v @ attn^T) + bp

Sharding: 8 cores = 2 batches x 4 query chunks of 1024. Each core receives
its batch's full image ROLLED so its local 1024 query columns come first,
making the SPMD program identical on every core (key order under softmax is
permutation invariant). GroupNorm + keys/values cover the full image on each
core; queries/residual only the local chunk.

Weight fusions (host-precomputed, all exact algebra):
  W2 = k_w^T (s q_w), b2 = k_w^T (s q_b):  scores^T = hn^T (W2 hn + b2)
       (the per-query offset bk.q is softmax-invariant and dropped) — K is
       never materialized.
  W3 = proj_w v_w:  since column scaling by 1/den commutes through left
       matmul, out = (W3 hn E) * recip_den — V and proj collapse into one.
  bp_eff = proj_w v_b + proj_b.

fp8 DoubleRow acceleration (tolerance is 2e-2; this lands ~4e-3):
  All four big matmul groups (q2, vt3, scores, AV) run as float8e4
  DoubleRow matmuls: contraction 256 deep per instruction at the same
  per-instruction cost as a 128-deep fp32r matmul (measured ~280 ns per
  [256]x[128x512] MM) — 2x the MAC rate, halving the MM count vs fp32r.
  Scale folding keeps everything in fp8's sweet spot and is exact:
    w28 = fp8(AW2*W2) so scores_psum = AW2*scores; exp applies scale 1/AW2.
    Per-key exp shift of EXP_SHIFT keeps E=exp(s+shift) <= ~40 < 240 (fp8e4
    max); the shift cancels between numerator and denominator.
    w38 = fp8(AV3*W3) so vt3/AV carry AV3; the ones-vector of the
    denominator cross-partition sum matmul is AV3 instead of 1, so the
    reciprocal cancels AV3 exactly.
  GroupNorm stats run on f32 x (resident, also reused for the residual);
  the normalize step writes hn directly as fp8 (never materialized f32).

Per-core dataflow (c = channel, j = key pos, i = query pos):
  hn8a/hn8b [c-part, 2-slab, n] fp8 — slab pairs (0,1) and (2,3) giving the
  256-deep DoubleRow contraction layout.
  q28 = fp8(W28 hn + AW2 b2)  [c-part, pair, slab, i-local]
  vt38 = fp8(hn^T W38^T)      [j-part, 16 chunk-pair, 2, c]
  per query tile of 512 (outU resident in 4 PSUM banks across the key loop):
    per key chunk PAIR of 256 (2 x 128):
      scoresT pair = hn8^T q28   [j-part, 2, i] one 2-bank PSUM tile
      E8 = exp(scale*psum+shift) ONE ScalarE activation over 1024 cols, fp8
      den partials                (DVE even / Pool odd pairs)
      outU += vt38^T E8          4 DoubleRow matmuls accumulating in PSUM
    y = outU * recip(AV3*den) + bp_eff + x_local   (x still resident)
~342 matmuls total vs 678 fp32r in the prior version.
"""

import numpy as np
import ml_dtypes

import concourse.bass as bass
import concourse.mybir as mybir
from concourse import bacc
import concourse.tile as tile
from concourse import bass_utils

P = 128          # partitions
C = 512          # channels
CS = C // P      # channel slabs (4)
G = 32           # groups
GS = C // G      # channels per group (16)
EPS = 1e-6
F32 = mybir.dt.float32
F32R = mybir.dt.float32r
FP8 = mybir.dt.float8e4
DR = mybir.MatmulPerfMode.DoubleRow
AL = mybir.AluOpType
AF = mybir.ActivationFunctionType

N_FULL = 4096    # spatial positions (64*64)
NQ = 1024        # local query chunk per core
IT = 512         # i-tile (queries per scores matmul free dim)

AW2 = 64.0       # fp8 range scale folded into W2 (and b2); exp scale 1/AW2
AV3 = 8.0        # fp8 range scale folded into W3; cancelled via ones=AV3
EXP_SHIFT = -2.0  # exp(s + shift): keeps E below fp8e4 max; cancels in ratio


def build_nc(n=N_FULL, nq=NQ, repeat=1, ablate=()):
    """Build the per-core Bass program. All 8 cores run this same program."""
    njc = n // P          # 32 key chunks
    npr = njc // 2        # 16 key chunk pairs
    nit = nq // IT        # 2 query tiles
    ablate = set(ablate)

    nc = bacc.Bacc("TRN2", target_bir_lowering=False, debug=False)

    x_d = nc.dram_tensor("x", [C, n], F32, kind="ExternalInput")
    w28_d = nc.dram_tensor("w28", [P, 2, 2, C], FP8, kind="ExternalInput")
    w38_d = nc.dram_tensor("w38", [P, 2, 2, C], FP8, kind="ExternalInput")
    # per partition p: [bq(CS), bp(CS), gamma(CS), beta(CS), bo_row(P), onv_row(P)]
    cpk_d = nc.dram_tensor("cpk", [P, 4 * CS + 2 * P], F32, kind="ExternalInput")
    y_d = nc.dram_tensor("y", [C, nq], F32, kind="ExternalOutput")

    # [C, n] viewed as [P, CS, n]: channel c = slab*128 + partition
    x_t = x_d.rearrange("(o p) n -> p o n", p=P)
    y_t = y_d.rearrange("(o p) n -> p o n", p=P)

    with tile.TileContext(nc) as tc:
        with (
            tc.tile_pool(name="consts", bufs=1) as consts,
            tc.tile_pool(name="xp", bufs=1) as xp,
            tc.tile_pool(name="hp", bufs=1) as hp,
            tc.tile_pool(name="persist", bufs=1) as persist,
            tc.tile_pool(name="ep", bufs=6) as ep,
            tc.tile_pool(name="dt", bufs=4) as dtp,
            tc.tile_pool(name="xres", bufs=1) as xres,
            tc.tile_pool(name="yp", bufs=4) as yp,
            tc.tile_pool(name="psmm", bufs=4, space="PSUM") as psmm,
            tc.tile_pool(name="psacc", bufs=4, space="PSUM") as psacc,
        ):
            # ---- constants (outside the repeat loop) ----
            w28_sb = consts.tile([P, 2, 2, C], FP8, tag="w28")
            w38_sb = consts.tile([P, 2, 2, C], FP8, tag="w38")
            nc.scalar.dma_start(out=w28_sb, in_=w28_d[:, :, :, :])
            nc.scalar.dma_start(out=w38_sb, in_=w38_d[:, :, :, :])
            cpk_sb = consts.tile([P, 4 * CS + 2 * P], F32, tag="cpk")
            nc.scalar.dma_start(out=cpk_sb, in_=cpk_d[:, :])
            bq_sb = cpk_sb[:, 0 * CS:1 * CS]
            bp_sb = cpk_sb[:, 1 * CS:2 * CS]
            gam_sb = cpk_sb[:, 2 * CS:3 * CS]
            bet_sb = cpk_sb[:, 3 * CS:4 * CS]
            bo_sb = cpk_sb[:, 4 * CS:4 * CS + P]
            onv_sb = cpk_sb[:, 4 * CS + P:4 * CS + 2 * P]
            e8c_sb = consts.tile([P, 2, IT], FP8, tag="e8c")
            nc.vector.memset(e8c_sb, 1.0)
            eps_sb = consts.tile([P, 1], F32, tag="eps")
            nc.vector.memset(eps_sb, EPS)
            sh_sb = consts.tile([P, 1], F32, tag="sh")
            nc.vector.memset(sh_sb, EXP_SHIFT)

            HN = n // 2   # x slabs live as two half tiles for DMA pipelining

            def body():
                # ---- phase 1: load x, groupnorm per slab (groups of 16
                # channels never cross a 128-channel slab). x stays resident
                # f32 (reused for the residual); hn is written as fp8 into
                # the DoubleRow slab-pair layout.
                x_sbs = []
                hn8 = [hp.tile([P, 2, n], FP8, tag=f"hn8{pr}", name=f"hn8{pr}")
                       for pr in range(2)]
                for po in range(CS):
                    parts = []
                    engs = [nc.sync, nc.gpsimd]
                    for hh in range(2):
                        xsh = xp.tile([P, HN], F32, tag=f"x{po}_{hh}",
                                      name=f"x{po}_{hh}")
                        engs[hh].dma_start(
                            out=xsh, in_=x_t[:, po, hh * HN:(hh + 1) * HN])
                        parts.append(xsh)
                    x_sbs.append(parts)
                    if "nogn" in ablate:
                        scl = consts.tile([P, 1], F32, tag=f"scl{po}",
                                          name=f"scl{po}")
                        shf = consts.tile([P, 1], F32, tag=f"shf{po}",
                                          name=f"shf{po}")
                        nc.vector.memset(scl, 1.0)
                        nc.vector.memset(shf, 0.0)
                        if "nonorm" not in ablate:
                            for hh in range(2):
                                nc.vector.tensor_scalar(
                                    out=hn8[po // 2][:, po % 2,
                                                     hh * HN:(hh + 1) * HN],
                                    in0=parts[hh], scalar1=scl, scalar2=shf,
                                    op0=AL.mult, op1=AL.subtract)
                        continue
                    nchunk = n // 512
                    nch = nchunk // 2
                    stats = consts.tile([P, nchunk, 6], F32, tag=f"st{po}",
                                        name=f"st{po}")
                    for hh in range(2):
                        xs3 = parts[hh].rearrange("p (s f) -> p s f", f=512)
                        for s in range(nch):
                            nc.vector.bn_stats(out=stats[:, hh * nch + s, :],
                                               in_=xs3[:, s, :])
                    mv = consts.tile([P, 2], F32, tag=f"mv{po}", name=f"mv{po}")
                    nc.vector.bn_aggr(out=mv, in_=stats)
                    # var -> E[x^2] = mean*mean + var (in place)
                    nc.vector.scalar_tensor_tensor(
                        out=mv[:, 1:2], in0=mv[:, 0:1], scalar=mv[:, 0:1],
                        in1=mv[:, 1:2], op0=AL.mult, op1=AL.add)
                    # group-average within the slab: [P, 2] = BO^T @ mv
                    ps_st = psmm.tile([P, IT], F32, tag="ps_mm",
                                      name="ps_st")
                    nc.tensor.matmul(ps_st[:, 0:2], bo_sb, mv,
                                     start=True, stop=True)
                    mvg = consts.tile([P, 2], F32, tag=f"mvg{po}",
                                      name=f"mvg{po}")
                    nc.vector.tensor_copy(out=mvg, in_=ps_st[:, 0:2])
                    gmean = mvg[:, 0:1]   # group E[x] per channel
                    gex2 = mvg[:, 1:2]    # group E[x^2] per channel
                    scl = consts.tile([P, 1], F32, tag=f"scl{po}",
                                      name=f"scl{po}")
                    shf = consts.tile([P, 1], F32, tag=f"shf{po}",
                                      name=f"shf{po}")
                    # scl <- -var = mean^2 - E[x^2]
                    nc.vector.scalar_tensor_tensor(
                        out=scl, in0=gmean, scalar=gmean, in1=gex2,
                        op0=AL.mult, op1=AL.subtract)
                    # sqrt(var + eps) via activation scale=-1
                    nc.scalar.activation(out=scl, in_=scl, func=AF.Sqrt,
                                         bias=eps_sb, scale=-1.0)
                    nc.vector.reciprocal(out=scl, in_=scl)
                    nc.vector.tensor_mul(out=scl, in0=scl,
                                         in1=gam_sb[:, po:po + 1])
                    # shf <- gmean*scl - beta = -(true shift)
                    nc.vector.scalar_tensor_tensor(
                        out=shf, in0=gmean, scalar=scl,
                        in1=bet_sb[:, po:po + 1], op0=AL.mult, op1=AL.subtract)
                    # hn8 = fp8(x*scl - shf), straight into slab-pair layout
                    for hh in ([] if "nonorm" in ablate else range(2)):
                        nc.vector.tensor_scalar(
                            out=hn8[po // 2][:, po % 2,
                                             hh * HN:(hh + 1) * HN],
                            in0=parts[hh], scalar1=scl, scalar2=shf,
                            op0=AL.mult, op1=AL.subtract)

                # ---- phase 2: q28 = fp8(W28 @ hn + AW2*b2) for local
                # queries. One 2-bank PSUM tile per output chunk cc, then a
                # single wide activation applies bias and quantizes.
                q28 = persist.tile([P, 2, 2, nq], FP8, tag="q28", name="q28")
                for cc in range(CS):
                    for it in range(nit):
                        isl = slice(it * IT, (it + 1) * IT)
                        psq = psmm.tile([P, IT], F32, tag="ps_mm", name="ps_q")
                        for pr in range(2):
                            nc.tensor.matmul(
                                psq,
                                w28_sb[:, pr, :, cc * P:(cc + 1) * P],
                                hn8[pr][:, :, isl],
                                start=(pr == 0), stop=(pr == 1),
                                perf_mode=DR)
                        nc.vector.tensor_scalar_add(
                            out=q28[:, cc // 2, cc % 2, isl],
                            in0=psq, scalar1=bq_sb[:, cc:cc + 1])
                # prefetch residual x slices for both query tiles now; they
                # complete during the attention phase
                xrs = {}
                for it in range(nit):
                    for cc in range(CS):
                        xr = xres.tile([P, IT], F32, tag=f"xr{it}{cc}",
                                       name=f"xr{it}{cc}")
                        nc.gpsimd.dma_start(
                            out=xr,
                            in_=x_t[:, cc, it * IT:(it + 1) * IT])
                        xrs[(it, cc)] = xr

                # ---- phase 2b: vt38 = fp8(hn^T W38^T), all key chunks,
                # DoubleRow layout [j-part, chunk-pair, 2, C] ----
                vt38 = persist.tile([P, npr, 2, C], FP8, tag="vt38",
                                    name="vt38")
                for jcg in range(njc):
                    psv = psmm.tile([P, IT], F32, tag="ps_mm", name="ps_v")
                    js = jcg * P
                    for pr in range(2):
                        nc.tensor.matmul(
                            psv,
                            hn8[pr][:, :, js:js + P],
                            w38_sb[:, pr, :, :],
                            start=(pr == 0), stop=(pr == 1),
                            perf_mode=DR)
                    if jcg % 2 == 0:
                        nc.vector.tensor_copy(
                            out=vt38[:, jcg // 2, jcg % 2, :], in_=psv)
                    else:
                        nc.scalar.copy(
                            out=vt38[:, jcg // 2, jcg % 2, :], in_=psv)

                # ---- phase 3: per query tile, one pass over all key chunk
                # pairs with the output accumulating in PSUM the whole way ----
                for it in range(nit):
                    isl = slice(it * IT, (it + 1) * IT)
                    dens = [persist.tile([P, IT], F32, tag=f"den{a}{it}",
                                         name=f"den{a}{it}")
                            for a in range(4)]
                    pos = [psacc.tile([P, IT], F32, tag="po", name=f"po{cc}")
                           for cc in range(CS)]
                    for t in range(npr):
                        pss2 = psmm.tile([P, IT], F32, tag="ps_mm",
                                         name="ps_s0")
                        pss3 = psmm.tile([P, IT], F32, tag="ps_mm",
                                         name="ps_s1")
                        pboth = [pss2, pss3]
                        if "noscore" not in ablate:
                            for q in range(2):
                                js = (2 * t + q) * P
                                for pr in range(2):
                                    nc.tensor.matmul(
                                        pboth[q],
                                        hn8[pr][:, :, js:js + P],
                                        q28[:, pr, :, isl],
                                        start=(pr == 0), stop=(pr == 1),
                                        perf_mode=DR)
                        else:
                            for q in range(2):
                                nc.tensor.matmul(
                                    pboth[q], hn8[0][:, :, 0:P],
                                    q28[:, 0, :, isl], start=True, stop=True,
                                    perf_mode=DR)
                        if "noexp2" in ablate:
                            e8 = e8c_sb
                        else:
                            e8 = ep.tile([P, 2, IT], FP8, tag="e", name="e")
                            for q in range(2):
                                nc.scalar.activation(
                                    out=e8[:, q, :], in_=pboth[q],
                                    func=(AF.Copy if "noexp" in ablate
                                          else AF.Exp),
                                    scale=1.0 / AW2, bias=sh_sb)
                        # denominator partials: 4 round-robin accumulators
                        # (2 DVE chains, 2 Pool chains); each chain is hit
                        # every 4th pair so it never gates the loop
                        if "noden" in ablate:
                            if t == 0:
                                for a in range(4):
                                    nc.vector.memset(dens[a], 1.0)
                        else:
                            eng = nc.vector if t % 2 == 0 else nc.gpsimd
                            deng = dens[t % 4]
                            if t < 4:
                                eng.tensor_tensor(out=deng, in0=e8[:, 0, :],
                                                  in1=e8[:, 1, :], op=AL.add)
                            else:
                                tmp = dtp.tile([P, IT], F32, tag="dt", name="dt")
                                eng.tensor_tensor(out=tmp, in0=e8[:, 0, :],
                                                  in1=e8[:, 1, :], op=AL.add)
                                eng.tensor_add(out=deng, in0=deng, in1=tmp)
                        # AV accumulate into psum across the whole key loop
                        if "noav" not in ablate or t in (0, npr - 1):
                            for cc in range(CS):
                                nc.tensor.matmul(
                                    pos[cc],
                                    vt38[:, t, :, cc * P:(cc + 1) * P],
                                    e8,
                                    start=(t == 0), stop=(t == npr - 1),
                                    perf_mode=DR)

                    # tail: y = pos*recip + bp_eff + x  (recip via one
                    # AV3-valued ones matmul: cross-partition sum + bcast)
                    nc.vector.tensor_add(out=dens[0], in0=dens[0], in1=dens[2])
                    nc.gpsimd.tensor_add(out=dens[1], in0=dens[1], in1=dens[3])
                    nc.vector.tensor_add(out=dens[0], in0=dens[0], in1=dens[1])
                    ps_d = psmm.tile([P, IT], F32, tag="ps_mm", name="ps_d")
                    nc.tensor.matmul(ps_d, onv_sb, dens[0],
                                     start=True, stop=True)
                    recip = consts.tile([P, IT], F32, tag=f"recip{it}",
                                        name=f"recip{it}")
                    nc.vector.reciprocal(out=recip, in_=ps_d)
                    for cc in range(CS):
                        yt = yp.tile([P, IT], F32, tag="yt", name="yt")
                        nc.vector.tensor_tensor(
                            out=yt, in0=pos[cc], in1=recip, op=AL.mult)
                        nc.vector.scalar_tensor_tensor(
                            out=yt, in0=yt, scalar=bp_sb[:, cc:cc + 1],
                            in1=xrs[(it, cc)], op0=AL.add, op1=AL.add)
                        nc.scalar.dma_start(out=y_t[:, cc, isl], in_=yt)

            if repeat == 1:
                body()
            else:
                with tc.For_i(0, repeat, 1):
                    body()

    nc.compile()
    return nc


_NC_CACHE = {}


def _get_nc(n=N_FULL, nq=NQ, repeat=1, ablate=()):
    key = (n, nq, repeat, tuple(sorted(ablate)))
    if key not in _NC_CACHE:
        _NC_CACHE[key] = build_nc(n, nq, repeat, ablate)
    return _NC_CACHE[key]


def _fp8(a):
    return np.clip(a, -240.0, 240.0).astype(ml_dtypes.float8_e4m3)


def make_in_maps(x, q_w, q_b, k_w, k_b, v_w, v_b, proj_w, proj_b,
                 norm_gamma, norm_beta, n_cores=8):
    """Build per-core input dicts from the full problem inputs."""
    B = x.shape[0]
    n = x.shape[2] * x.shape[3]
    xf = np.ascontiguousarray(x.reshape(B, C, n).astype(np.float32))
    scale = np.float64(C) ** -0.5
    # fused score projection: scores^T = hn^T @ (W2 hn + b2) (+ const per
    # query, dropped — softmax invariant)
    W2 = k_w.astype(np.float64).T @ (q_w.astype(np.float64) * scale)
    b2 = k_w.astype(np.float64).T @ (q_b.astype(np.float64) * scale)
    W3 = proj_w.astype(np.float64) @ v_w.astype(np.float64)
    bp_eff = (proj_w.astype(np.float64) @ v_b.astype(np.float64)
              + proj_b.astype(np.float64)).astype(np.float32)

    def pack_dr(W, s):  # [C_out, C_in] -> fp8 [P, 2, 2, C_out], scaled
        Wt = np.ascontiguousarray((W.T * s).astype(np.float32))  # [C_in, C_out]
        return _fp8(Wt.reshape(2, 2, P, C).transpose(2, 0, 1, 3))

    w28 = np.ascontiguousarray(pack_dr(W2, AW2))
    w38 = np.ascontiguousarray(pack_dr(W3, AV3))
    # block-diagonal group-averaging matrix: 16x16 blocks of 1/16
    bo = np.zeros((P, P), np.float32)
    for g in range(P // GS):
        bo[g * GS:(g + 1) * GS, g * GS:(g + 1) * GS] = 1.0 / GS
    onv = np.full((P, P), AV3, np.float32)

    def r2h(v):  # [C] -> [P, CS] with c = o*P + p
        return np.ascontiguousarray(
            np.asarray(v, np.float64).reshape(CS, P).T.astype(np.float32))
    cpk = np.concatenate(
        [r2h(b2 * AW2), r2h(bp_eff),
         r2h(norm_gamma.astype(np.float32)), r2h(norm_beta.astype(np.float32)),
         bo, onv], axis=1)
    chunks = n_cores // B
    nq = n // chunks
    in_maps = []
    for g in range(n_cores):
        b, qc = divmod(g, chunks)
        xg = np.roll(xf[b], -qc * nq, axis=1)
        in_maps.append(dict(
            x=np.ascontiguousarray(xg), w28=w28, w38=w38, cpk=cpk))
    return in_maps


def kernel(**inputs):
    x = np.asarray(inputs["x"], np.float32)
    B, _, H, W = x.shape
    n = H * W
    chunks = 8 // B
    nq = n // chunks
    in_maps = make_in_maps(
        x, np.asarray(inputs["q_w"]), np.asarray(inputs["q_b"]),
        np.asarray(inputs["k_w"]), np.asarray(inputs["k_b"]),
        np.asarray(inputs["v_w"]), np.asarray(inputs["v_b"]),
        np.asarray(inputs["proj_w"]), np.asarray(inputs["proj_b"]),
        np.asarray(inputs["norm_gamma"]), np.asarray(inputs["norm_beta"]))
    nc = _get_nc(n, nq)
    res = bass_utils.run_bass_kernel_spmd(nc, in_maps, core_ids=list(range(8)))
    y = np.empty((B, C, n), np.float32)
    for g in range(8):
        b, qc = divmod(g, chunks)
        y[b][:, qc * nq:(qc + 1) * nq] = res.results[g]["y"]
    return y.reshape(B, C, H, W)
